# revision 19
# baseline (speedup 1.0000x reference)
"""Fused multi-head-attention Bass kernel for Trainium2, batch-parallel over 8 cores.

Reference computation (per batch element b):
    qkv = x @ w_qkv + b_qkv            # [T, 2304]
    q, k, v = split(qkv, 3)            # [T, 768] each (full-width heads, no head split)
    s = q @ k.T / sqrt(64)             # [T, T]
    a = softmax(s, axis=-1)
    y = (a @ v) @ w_out + b_out        # [T, 768]

Sharding: data-parallel over batch (B=8 -> 8 NeuronCores), zero collectives.

Algebraic restructuring (host precomputes, fp64):
    M  = Wq @ Wk.T / 8      [768, 768]
    NM = Wv @ W_out         [768, 768]
    h  = Wk @ bq / 8        [768]
    r  = bv @ W_out + b_out [768]
  Then s/8 = (x@M) x^T + (x@(Wk bq^T)/8 per-key) + per-query-const (cancels in
  softmax) + const, so with G' = x@M + 1 h^T:
    sT[k, q] = sum_d x[k,d] * G'[q,d]   (exactly softmax-equivalent scores)
    y = softmax-weighted average of z = x@NM, plus row-const r.
  This removes the q/k/v and output projections entirely: per-core matmul work
  drops from 688k PE-rows (11.3 GMAC) to ~546k (8.9 GMAC).

Per-core layout (T=2048, D=768):
  - xT [d, t] fp16 resident; G'T [d, t] fp16 (computed on device, h folded
    into the eviction bias); z [t, e] bf16 resident. fp16 keeps the score
    path's quantization noise 8x below bf16 at the same PE rate (measured:
    fp32r / fp16 / bf16 all run ~1 row/cycle on HW for these shapes).
  - sT[tk, tq] = xT-slices (stationary) x G'T (moving): softmax reduction dim
    tk lands on partitions; scores are fp16-operand/fp32-accumulate exact.
  - exp fused into the PSUM->SBUF eviction (ScalarE), bf16 output. No
    max-subtraction needed: |s/8| <= ~20, exp fits fp32/bf16 range.
  - denominator: DVE partial sums + ones-vector matmul + PE transpose + DVE
    reciprocal; emitted before the o-groups whose evictions read recip.
  - o[tq, e] = exp-slices (stationary, bf16) x z (moving, bf16); the two
    384-wide e-chunks interleave so each stationary weight load feeds two
    matmuls. Eviction: 1/denom per-partition scale on ScalarE, +r on DVE,
    per-chunk DMA out.
  - Phase order: GT (chunk-streaming, overlapped with the input DMAs) ->
    scores block 0 -> z projection -> per-block denominator + output, which
    hides the z DMA and the block-0 denominator latency under PE work.
  - Measured: HW 225-253 us/core across runs (cost model 244.7 us; ~92% PE
    busy), rel err 3.4e-3 vs the fp32 reference (baseline fused kernel that
    computes the projections explicitly: 324.5 us, PE-bound at fp32r).
"""

import numpy as np

import concourse.bacc as bacc
import concourse.bass as bass
import concourse.mybir as mybir
import concourse.tile as tile
from concourse import bass_utils

F32 = mybir.dt.float32
F32R = mybir.dt.float32r
F16 = mybir.dt.float16
BF16 = mybir.dt.bfloat16
AF = mybir.ActivationFunctionType

B = 8
T = 2048
D = 768
ND = D // 128          # 6 contraction tiles
NT = T // 128          # 16 sequence tiles
NE = ND                # kept for test.py compat
TQB = 512              # query-block width
NBLK = T // TQB        # 4 query blocks
ECH = 384              # e-chunk width for z / o matmuls (fits one PSUM bank)


def _build_program(nc, reps=1):
    xT_d = nc.dram_tensor("xT", [D, T], F16, kind="ExternalInput").ap()
    m_d = nc.dram_tensor("m_mat", [D, D], F16, kind="ExternalInput").ap()
    nm_d = nc.dram_tensor("nm_mat", [D, D], F16, kind="ExternalInput").ap()
    h_d = nc.dram_tensor("h_pt", [128, ND], F32, kind="ExternalInput").ap()
    r_d = nc.dram_tensor("r_bcast", [128, D], F32, kind="ExternalInput").ap()
    ones_d = nc.dram_tensor("ones", [128, 128], F32R, kind="ExternalInput").ap()
    y_d = nc.dram_tensor("y", [T, D], F32, kind="ExternalOutput").ap()

    with tile.TileContext(nc) as tc:
        for _ in range(reps):
            _emit(tc, nc, xT_d, m_d, nm_d, h_d, r_d, ones_d, y_d)
    nc.compile()


def _emit(tc, nc, xT_d, m_d, nm_d, h_d, r_d, ones_d, y_d):
    with (
        tc.tile_pool(name="const", bufs=1) as cp,
        tc.tile_pool(name="resident", bufs=1) as rp,
        tc.tile_pool(name="ps", bufs=4, space="PSUM") as pp,
    ):
        ones = cp.tile([128, 128], F32R)
        hb = cp.tile([128, ND], F32)
        rb = cp.tile([128, D], F32)
        recip = cp.tile([128, NT], F32)

        xT = rp.tile([128, ND, T], F16)
        GT = rp.tile([128, ND, T], F16)
        z = rp.tile([128, NT, D], BF16)

        with (
            tc.tile_pool(name="wmat", bufs=1) as wp,
            tc.tile_pool(name="exp", bufs=NT) as ep,
            tc.tile_pool(name="yrow", bufs=3) as yp,
            tc.tile_pool(name="dn", bufs=2) as dnp,
        ):
            m_t = wp.tile([128, ND, D], F16)
            nm_t = wp.tile([128, ND, D], F16)
            # startup order: GT runs chunk-streaming (n outer), so it needs
            # xT chunk 0 + M first; later chunks arrive under compute. NM is
            # only needed for the z projection which is emitted after block
            # 0's scores, giving the DMA engine plenty of slack.
            nc.sync.dma_start(
                xT[:, :, 0:512], xT_d[:, 0:512].rearrange("(j p) t -> p j t", p=128)
            )
            nc.sync.dma_start(
                m_t[:, :, 0:128], m_d[:, 0:128].rearrange("(j p) e -> p j e", p=128)
            )
            nc.sync.dma_start(hb[:], h_d[:])
            nc.sync.dma_start(
                m_t[:, :, 128:D], m_d[:, 128:D].rearrange("(j p) e -> p j e", p=128)
            )
            for n in range(1, T // 512):
                nc.sync.dma_start(
                    xT[:, :, n * 512 : (n + 1) * 512],
                    xT_d[:, n * 512 : (n + 1) * 512].rearrange("(j p) t -> p j t", p=128),
                )
            nc.sync.dma_start(nm_t[:], nm_d.rearrange("(j p) e -> p j e", p=128))
            nc.sync.dma_start(ones[:], ones_d[:])
            nc.sync.dma_start(rb[:], r_d[:])

            # ---- G'T[e-tile, t] = M^T x + h: stationary M-tile, moving xT ----
            for n in range(T // 512):
                for e in range(ND):
                    ps = pp.tile([128, 512], F32, tag="ps")
                    for jd in range(ND):
                        nc.tensor.matmul(
                            ps[:],
                            m_t[:, jd, e * 128 : (e + 1) * 128],
                            xT[:, jd, n * 512 : (n + 1) * 512],
                            start=(jd == 0),
                            stop=(jd == ND - 1),
                        )
                    nc.scalar.activation(
                        GT[:, e, n * 512 : (n + 1) * 512],
                        ps[:],
                        AF.Identity,
                        bias=hb[:, e : e + 1],
                    )

            def emit_scores(blk):
                # scores^T tiles + fused exp; bf16 out
                exps = []
                for i in range(NT):
                    ps = pp.tile([128, TQB], F32, tag="ps")
                    for jd in range(ND):
                        nc.tensor.matmul(
                            ps[:],
                            xT[:, jd, i * 128 : (i + 1) * 128],
                            GT[:, jd, blk * TQB : (blk + 1) * TQB],
                            start=(jd == 0),
                            stop=(jd == ND - 1),
                        )
                    ex = ep.tile([128, TQB], BF16, tag="exp")
                    nc.scalar.activation(ex[:], ps[:], AF.Exp)
                    exps.append(ex)

                # softmax denominator partial sums on DVE
                dacc = dnp.tile([128, TQB], F32R, tag="dacc")
                nc.vector.tensor_add(dacc[:], exps[0][:], exps[1][:])
                for i in range(2, NT):
                    nc.vector.tensor_add(dacc[:], dacc[:], exps[i][:])
                return exps, dacc

            def emit_denom(blk, dacc):
                # cross-partition denominator reduction. Must be emitted
                # before the o-group evictions that read recip (tile deps
                # follow emission order).
                dn_ps = pp.tile([1, TQB], F32, tag="ps")
                nc.tensor.matmul(dn_ps[:], ones[:, 0:1], dacc[:], start=True, stop=True)
                dn = dnp.tile([1, TQB], F32)
                nc.vector.tensor_copy(dn[:], dn_ps[:])
                dnpt_ps = pp.tile([128, TQB // 128], F32, tag="ps")
                for l2 in range(TQB // 128):
                    nc.tensor.transpose(
                        dnpt_ps[:, l2 : l2 + 1],
                        dn[0:1, l2 * 128 : (l2 + 1) * 128],
                        ones[0:1, 0:1].bitcast(F32),
                    )
                nc.vector.reciprocal(
                    recip[:, blk * (TQB // 128) : (blk + 1) * (TQB // 128)],
                    dnpt_ps[:],
                )

            def emit_out(blk, exps):
                # o[tq, e] = exp-slices (stationary) x z (moving), normalized
                # by 1/denom on ScalarE, +r on DVE, DMA per 384-chunk. The two
                # e-chunks interleave so each exp stationary feeds two
                # consecutive matmuls (amortizes the weight load).
                for l in range(TQB // 128):
                    g = blk * (TQB // 128) + l
                    yt = yp.tile([128, D], F32)
                    psc = [
                        pp.tile([128, ECH], F32, tag="ys", bufs=4, name=f"ops{blk}_{l}_{c}")
                        for c in range(D // ECH)
                    ]
                    for i in range(NT):
                        st = exps[i][:, l * 128 : (l + 1) * 128]
                        for c in range(D // ECH):
                            nc.tensor.matmul(
                                psc[c][:],
                                st,
                                z[:, i, c * ECH : (c + 1) * ECH],
                                start=(i == 0),
                                stop=(i == NT - 1),
                            )
                    for c in range(D // ECH):
                        ysl = yt[:, c * ECH : (c + 1) * ECH]
                        nc.scalar.activation(
                            ysl, psc[c][:], AF.Identity, scale=recip[:, g : g + 1]
                        )
                        nc.vector.tensor_add(ysl, ysl, rb[:, c * ECH : (c + 1) * ECH])
                        nc.sync.dma_start(
                            y_d[g * 128 : (g + 1) * 128, c * ECH : (c + 1) * ECH], ysl
                        )

            # block 0 scores immediately after GT (z is not needed yet);
            # the z projection then runs while block 0's denominator settles
            exps0, dacc0 = emit_scores(0)

            # ---- z[t-tile, e] = x @ NM (bf16): stationary xT-slice, e-chunks
            # interleaved for stationary reuse ----
            for i in range(NT):
                psc = [
                    pp.tile([128, ECH], F32, tag="ys", bufs=4, name=f"zps{i}_{c}")
                    for c in range(D // ECH)
                ]
                for jd in range(ND):
                    st = xT[:, jd, i * 128 : (i + 1) * 128]
                    for c in range(D // ECH):
                        nc.tensor.matmul(
                            psc[c][:],
                            st,
                            nm_t[:, jd, c * ECH : (c + 1) * ECH],
                            start=(jd == 0),
                            stop=(jd == ND - 1),
                        )
                for c in range(D // ECH):
                    nc.vector.tensor_copy(z[:, i, c * ECH : (c + 1) * ECH], psc[c][:])

            emit_denom(0, dacc0)
            emit_out(0, exps0)
            for blk in range(1, NBLK):
                exps, dacc = emit_scores(blk)
                emit_denom(blk, dacc)
                emit_out(blk, exps)


_NC_CACHE = None


def build_nc(reps=1):
    nc = bacc.Bacc("TRN2", target_bir_lowering=False, debug=False)
    _build_program(nc, reps=reps)
    return nc


def _get_nc():
    global _NC_CACHE
    if _NC_CACHE is None:
        _NC_CACHE = build_nc(1)
    return _NC_CACHE


def _host_precompute(w_qkv, b_qkv, w_out, b_out):
    Wq = w_qkv[:, :D].astype(np.float64)
    Wk = w_qkv[:, D : 2 * D].astype(np.float64)
    Wv = w_qkv[:, 2 * D :].astype(np.float64)
    bq = b_qkv[:D].astype(np.float64)
    bv = b_qkv[2 * D :].astype(np.float64)
    M = (Wq @ Wk.T / 8.0).astype(np.float32)
    NM = (Wv @ w_out.astype(np.float64)).astype(np.float32)
    h = (Wk @ bq / 8.0).astype(np.float32)
    r = (bv @ w_out.astype(np.float64) + b_out.astype(np.float64)).astype(np.float32)
    h_pt = np.ascontiguousarray(h.reshape(ND, 128).T)
    r_bcast = np.ascontiguousarray(np.broadcast_to(r.reshape(1, D), (128, D)))
    return M, NM, h_pt, r_bcast


def kernel(x, w_qkv, b_qkv, w_out, b_out):
    x = np.asarray(x, dtype=np.float32)
    w_qkv = np.asarray(w_qkv, dtype=np.float32)
    b_qkv = np.asarray(b_qkv, dtype=np.float32)
    w_out = np.asarray(w_out, dtype=np.float32)
    b_out = np.asarray(b_out, dtype=np.float32)

    M, NM, h_pt, r_bcast = _host_precompute(w_qkv, b_qkv, w_out, b_out)
    ones_arr = np.ones((128, 128), dtype=np.float32)

    nc = _get_nc()
    in_maps = []
    for c in range(B):
        in_maps.append(
            {
                "xT": np.ascontiguousarray(x[c].T).astype(np.float16),
                "m_mat": M.astype(np.float16),
                "nm_mat": NM.astype(np.float16),
                "h_pt": h_pt,
                "r_bcast": r_bcast,
                "ones": ones_arr,
            }
        )

    try:
        res = bass_utils.run_bass_kernel_spmd(nc, in_maps, core_ids=list(range(B)))
    except Exception:
        # transient device hiccups (e.g. NRT exec-unit errors from a prior
        # wedged session) usually clear on retry
        res = bass_utils.run_bass_kernel_spmd(nc, in_maps, core_ids=list(range(B)))
    return np.stack([res.results[c]["y"] for c in range(B)], axis=0)
